# revision 31
# baseline (speedup 1.0000x reference)
"""MoE (dropless, top-2 of 8 experts, GLU erf-gelu MLP) Trainium2 kernel.

Expert-parallel across 8 NeuronCores: core c holds expert c's weights.
Each core:
  A. routes all T=4096 tokens (fp32 router matmul on PE-transposed x,
     batched softmax/top-2 on DVE/ACT),
  B. computes each token's compaction rank (free-dim scan + triangular-
     matrix matmul prefix over partitions), then builds slot->(tokid,
     weight, hit) tables with one-hot eq-matrix matmuls into PSUM,
  C. indirect-gathers the <=CPAD routed token rows from x, PE-transposes
     them, runs the GLU MLP with float32r matmuls (full PE speed on fp32
     data), multiplies rows by routing weight and adds bias/2 (each token
     is scattered by exactly TOP_K=2 cores), then indirect-scatters them
     into a dense [T, D] partial output (runtime zero-initializes it).
The host sums the 8 partial outputs.

Self-contained: hardcodes all shapes (x [2,2048,1024], E=8, F=2816).
"""

import os
import sys

import numpy as np

for _p in ("/opt/trn_rl_repo", "/root/.axon_site/_ro/trn_rl_repo"):
    if os.path.isdir(_p) and _p not in sys.path:
        sys.path.append(_p)

import concourse.bass as bass  # noqa: E402
import concourse.bacc as bacc  # noqa: E402
import concourse.mybir as mybir  # noqa: E402
import concourse.tile as tile  # noqa: E402
from concourse.bass import ds, ts  # noqa: E402
from concourse.masks import make_identity  # noqa: E402

F32 = mybir.dt.float32
F32R = mybir.dt.float32r
I32 = mybir.dt.int32
AF = mybir.ActivationFunctionType
OP = mybir.AluOpType

P = 128
T = 4096          # tokens (2*2048)
D = 1024          # model dim
F = 2816          # ffn dim
E = 8             # experts
NT = T // P       # 32 token tiles
DO = D // P       # 8 d-blocks
CPAD = 1280       # per-expert token capacity (avg load 1024, max seen 1091)
NJ = CPAD // P    # 10 slot tiles
FC = 256          # F chunk size
NFC = F // FC     # 11 chunks
FU = FC // P      # 2 subchunks of 128
CGRP = 2          # F chunks per PSUM accumulation group for y

# token blocks (moving dim of the h matmuls); f32r needs N>=256 for speed
TBLOCKS = []
_o = 0
while _o < CPAD:
    _b = min(512, CPAD - _o)
    TBLOCKS.append((_o, _b))
    _o += _b


def build_nc():
    nc = bacc.Bacc()

    x_d = nc.dram_tensor("x", [T, D], F32, kind="ExternalInput")
    rw_d = nc.dram_tensor("rw", [D, E], F32, kind="ExternalInput")
    w1_d = nc.dram_tensor("w1", [D, F], F32R, kind="ExternalInput")
    v1_d = nc.dram_tensor("v1", [D, F], F32R, kind="ExternalInput")
    w2_d = nc.dram_tensor("w2", [F, D], F32R, kind="ExternalInput")
    onehot_d = nc.dram_tensor("onehot", [P, E], F32, kind="ExternalInput")
    lstrict_d = nc.dram_tensor("lstrict", [P, P], F32, kind="ExternalInput")
    tokid_d = nc.dram_tensor("tokid", [P, NT], F32, kind="ExternalInput")
    slotiota_d = nc.dram_tensor("slotiota", [P, CPAD], F32, kind="ExternalInput")
    biasbg_d = nc.dram_tensor("biasbg", [P, D], F32, kind="ExternalInput")
    y_d = nc.dram_tensor("y", [T + P, D], F32, kind="ExternalOutput")

    with tile.TileContext(nc) as tc:
        with tc.tile_pool(name="persist", bufs=1) as pp:
            identity = pp.tile([P, P], F32)
            make_identity(nc, identity)
            lstrict = pp.tile([P, P], F32)
            nc.sync.dma_start(lstrict[:], lstrict_d[:])
            tokid = pp.tile([P, NT], F32)
            nc.sync.dma_start(tokid[:], tokid_d[:])
            onehot = pp.tile([P, E], F32)
            nc.sync.dma_start(onehot[:], onehot_d[:])
            slotiota = pp.tile([P, CPAD], F32)
            nc.sync.dma_start(slotiota[:], slotiota_d[:])
            rw_sb = pp.tile([P, DO, E], F32)
            nc.sync.dma_start(rw_sb[:], rw_d.rearrange("(o p) e -> p o e", p=P))
            biasbg = pp.tile([P, D], F32)
            nc.sync.dma_start(biasbg[:], biasbg_d[:])

            xgT = pp.tile([P, DO, CPAD], F32R)
            y_sb = pp.tile([P, NJ, D], F32)
            gidx_g = pp.tile([P, NJ], I32)   # gather: tokid*hit (0 if empty)
            gidx_s = pp.tile([P, NJ], I32)   # scatter: tokid + (1-hit)*T
            wslot = pp.tile([P, NJ], F32)

            _wcm = tc.tile_pool(name="wts", bufs=2)
            wpool = _wcm.__enter__()

            # ---------------- Phase A/B: routing + compaction ----------
            with (
                tc.tile_pool(name="xio", bufs=3) as xpool,
                tc.tile_pool(name="xt", bufs=2) as xtpool,
                tc.tile_pool(name="smx", bufs=1) as smx,
                tc.tile_pool(name="eqp", bufs=2) as eqp,
                tc.tile_pool(name="psAB", bufs=2, space="PSUM") as psAB,
            ):
                logits_all = smx.tile([P, NT, E], F32)

                GT = 2  # token-tiles per router matmul group
                for g in range(NT // GT):
                    xTg = xtpool.tile([P, DO, GT * P], F32, name="xTg")
                    for lf in range(GT):
                        f = g * GT + lf
                        x_t = xpool.tile([P, D], F32, name="x_t")
                        nc.sync.dma_start(x_t[:], x_d[ts(f, P), :])
                        for ob in range(0, DO, 4):
                            ps_tr = psAB.tile([P, 512], F32, tag="tr", bufs=4, name="ps_tr")
                            for oi in range(4):
                                nc.tensor.transpose(
                                    ps_tr[:, ts(oi, P)], x_t[:, ts(ob + oi, P)],
                                    identity[:],
                                )
                            dst = xTg[:, ob : ob + 4, ts(lf, P)]
                            if ob == 0:
                                nc.scalar.copy(dst, ps_tr[:].rearrange("p (o q) -> p o q", o=4))
                            else:
                                nc.vector.tensor_copy(dst, ps_tr[:].rearrange("p (o q) -> p o q", o=4))
                    # logitsT[e, tok] for GT*P tokens in one N=512 group
                    ps_lgT = psAB.tile([E, GT * P], F32, tag="lgT", name="ps_lgT")
                    for o in range(DO):
                        nc.tensor.matmul(
                            ps_lgT[:], rw_sb[:, o, :], xTg[:, o, :],
                            start=(o == 0), stop=(o == DO - 1),
                        )
                    lgT_sb = xtpool.tile([E, GT * P], F32, name="lgT_sb")
                    nc.scalar.copy(lgT_sb[:], ps_lgT[:])
                    for lf in range(GT):
                        f = g * GT + lf
                        ps_tr8 = psAB.tile([P, E], F32, tag="lgT", name="ps_tr8")
                        nc.tensor.transpose(
                            ps_tr8[:], lgT_sb[:, ts(lf, P)], identity[:E, :E]
                        )
                        nc.scalar.copy(logits_all[:, f, :], ps_tr8[:])

                # softmax + top-2 (batched over all tiles)
                m1 = smx.tile([P, NT], F32)
                nc.vector.reduce_max(m1[:, :, None], logits_all[:], axis=mybir.AxisListType.X)
                m1b = m1[:, :, None].to_broadcast([P, NT, E])
                shifted = smx.tile([P, NT, E], F32)
                nc.vector.tensor_tensor(shifted[:], logits_all[:], m1b, op=OP.subtract)
                exp_all = smx.tile([P, NT, E], F32)
                nc.scalar.activation(exp_all[:], shifted[:], AF.Exp)
                sumexp = smx.tile([P, NT], F32)
                nc.vector.reduce_sum(sumexp[:, :, None], exp_all[:], axis=mybir.AxisListType.X)
                recip = smx.tile([P, NT], F32)
                nc.vector.reciprocal(recip[:], sumexp[:])

                ismax = smx.tile([P, NT, E], F32)
                nc.vector.tensor_tensor(ismax[:], logits_all[:], m1b, op=OP.is_ge)
                nc.vector.tensor_scalar(ismax[:], ismax[:], -1e30, None, op0=OP.mult)
                masked = smx.tile([P, NT, E], F32)
                nc.vector.tensor_tensor(masked[:], logits_all[:], ismax[:], op=OP.add)
                m2 = smx.tile([P, NT], F32)
                nc.vector.reduce_max(m2[:, :, None], masked[:], axis=mybir.AxisListType.X)

                selt = smx.tile([P, NT, E], F32)
                ohb = onehot[:, None, :].to_broadcast([P, NT, E])
                nc.vector.tensor_tensor(selt[:], logits_all[:], ohb, op=OP.mult)
                sel = smx.tile([P, NT], F32)
                nc.vector.reduce_sum(sel[:, :, None], selt[:], axis=mybir.AxisListType.X)

                selsh = smx.tile([P, NT], F32)
                nc.vector.tensor_tensor(selsh[:], sel[:], m1[:], op=OP.subtract)
                expsel = smx.tile([P, NT], F32)
                nc.scalar.activation(expsel[:], selsh[:], AF.Exp)

                mask = smx.tile([P, NT], F32)
                wtok = smx.tile([P, NT], F32)
                nc.vector.tensor_tensor(mask[:], sel[:], m2[:], op=OP.is_ge)
                nc.vector.tensor_tensor(wtok[:], expsel[:], recip[:], op=OP.mult)
                nc.vector.tensor_tensor(wtok[:], wtok[:], mask[:], op=OP.mult)

                # rank = exclusive prefix of mask over token order (p-major):
                # free-dim scan within partition + Lstrict matmul across
                zero32 = smx.tile([P, NT], F32)
                nc.gpsimd.memset(zero32[:], 0.0)
                incl = smx.tile([P, NT], F32)
                nc.vector.tensor_tensor_scan(
                    incl[:], mask[:], zero32[:], 0.0, op0=OP.add, op1=OP.add
                )
                ps_base = psAB.tile([P, 4], F32, tag="cmp", name="ps_base")[:, 0:1]
                nc.tensor.matmul(
                    ps_base[:], lstrict[:], incl[:, NT - 1 : NT], start=True, stop=True
                )
                base = smx.tile([P, 1], F32)
                nc.scalar.copy(base[:], ps_base[:])
                exr = smx.tile([P, NT], F32)
                nc.vector.tensor_tensor(exr[:], incl[:], mask[:], op=OP.subtract)
                nc.vector.tensor_tensor(
                    exr[:], exr[:], base[:].to_broadcast([P, NT]), op=OP.add
                )
                # mexf = mask ? rank : CPAD, clamped to CPAD (overflow-safe)
                mexf = smx.tile([P, NT], F32)
                nc.vector.tensor_tensor(mexf[:], exr[:], mask[:], op=OP.mult)
                bigt = smx.tile([P, NT], F32)
                nc.vector.tensor_scalar(
                    bigt[:], mask[:], -float(CPAD), float(CPAD),
                    op0=OP.mult, op1=OP.add,
                )
                nc.vector.tensor_tensor(mexf[:], mexf[:], bigt[:], op=OP.add)
                nc.vector.tensor_scalar(mexf[:], mexf[:], float(CPAD), None, op0=OP.min)

                # slot tables: for slot-tile j, psum[m, 0:3] accumulates
                # (tokid, wtok, 1) of the token whose rank == j*128+m
                vals = smx.tile([P, NT, 3], F32)
                nc.vector.tensor_copy(vals[:, :, 0], tokid[:])
                nc.vector.tensor_copy(vals[:, :, 1], wtok[:])
                nc.vector.tensor_scalar(
                    vals[:, :, 2], mask[:], 0.0, 1.0, op0=OP.mult, op1=OP.add
                )
                FQ = 4  # token-tiles per eq compare
                for j in range(NJ):
                    ps_cmp = psAB.tile([P, 4], F32, tag="cmp", name="ps_cmp")
                    for f0 in range(0, NT, FQ):
                        eqm = eqp.tile([P, FQ, P], F32, tag="eq", name="eqm")
                        nc.vector.tensor_tensor(
                            eqm[:],
                            mexf[:, f0 : f0 + FQ, None].to_broadcast([P, FQ, P]),
                            slotiota[:, None, ts(j, P)].to_broadcast([P, FQ, P]),
                            op=OP.is_equal,
                        )
                        for q in range(FQ):
                            nc.tensor.matmul(
                                ps_cmp[:, 0:3],
                                eqm[:, q, :], vals[:, f0 + q, :],
                                start=(f0 == 0 and q == 0),
                                stop=(f0 + q == NT - 1),
                            )
                    gtmp = eqp.tile([P, 1], F32, tag="gtmp", name="gtmp")
                    # scatter idx = tokid*hit + (1-hit)*T
                    nc.vector.tensor_scalar(
                        gtmp[:], ps_cmp[:, 2:3], -float(T), float(T),
                        op0=OP.mult, op1=OP.add,
                    )
                    nc.vector.tensor_tensor(
                        gtmp[:], gtmp[:], ps_cmp[:, 0:1], op=OP.add
                    )
                    nc.vector.tensor_copy(gidx_s[:, j : j + 1], gtmp[:])
                    nc.vector.tensor_copy(gidx_g[:, j : j + 1], ps_cmp[:, 0:1])
                    nc.vector.tensor_copy(wslot[:, j : j + 1], ps_cmp[:, 1:2])

                # gather routed token rows; transpose to [d, slot]
                # (inside the A/B scope so it overlaps the compaction tail)
                for j in range(NJ):
                    xg_sb = xpool.tile([P, D], F32, tag="x_t", name="xg_sb")
                    nc.gpsimd.indirect_dma_start(
                        out=xg_sb[:],
                        out_offset=None,
                        in_=x_d[:],
                        in_offset=bass.IndirectOffsetOnAxis(
                            ap=gidx_g[:, j : j + 1], axis=0
                        ),
                    )
                    for ob in range(0, DO, 4):
                        ps_tr = psAB.tile([P, 512], F32, tag="tr", bufs=4, name="ps_tr2")
                        for oi in range(4):
                            nc.tensor.transpose(
                                ps_tr[:, ts(oi, P)], xg_sb[:, ts(ob + oi, P)],
                                identity[:],
                            )
                        dst = xgT[:, ob : ob + 4, ts(j, P)]
                        if ob == 0:
                            nc.scalar.copy(dst, ps_tr[:].rearrange("p (o q) -> p o q", o=4))
                        else:
                            nc.vector.tensor_copy(dst, ps_tr[:].rearrange("p (o q) -> p o q", o=4))

            # ---------------- Phase C: expert GLU MLP -------------------
            with (
                tc.tile_pool(name="hp", bufs=2) as hpool,
                tc.tile_pool(name="gl", bufs=2) as gpool,
                tc.tile_pool(name="psC", bufs=2, space="PSUM") as psC,
            ):
                # stream weights once (in chunk pairs); y accumulates in
                # PSUM across the pair, then adds into SBUF
                for cp in range(0, NFC, CGRP):
                    cs = [c for c in range(cp, min(cp + CGRP, NFC))]
                    hts = {}
                    w2s = {}
                    for c in cs:
                        w1c = wpool.tile([P, DO, FC], F32R, tag="w1", name="w1c")
                        nc.sync.dma_start(
                            w1c[:],
                            w1_d[:, ts(c, FC)].rearrange("(o p) f -> p o f", p=P),
                        )
                        v1c = wpool.tile([P, DO, FC], F32R, tag="v1", name="v1c")
                        nc.sync.dma_start(
                            v1c[:],
                            v1_d[:, ts(c, FC)].rearrange("(o p) f -> p o f", p=P),
                        )
                        w2s[c] = wpool.tile([P, FU, D], F32R, tag="w2", bufs=3, name="w2c")
                        nc.sync.dma_start(
                            w2s[c][:],
                            w2_d[ts(c, FC), :].rearrange("(u p) d -> p u d", p=P),
                        )
                        hts[c] = hpool.tile([P, FU, CPAD], F32R, bufs=3, name="hT")
                        for u in range(FU):
                            for (b0, bs) in TBLOCKS:
                                ph1 = psC.tile([P, 512], F32, tag="h1", name="ph1")
                                ph2 = psC.tile([P, 512], F32, tag="h2", name="ph2")
                                for o in range(DO):
                                    nc.tensor.matmul(
                                        ph1[:, :bs],
                                        w1c[:, o, ts(u, P)],
                                        xgT[:, o, ds(b0, bs)],
                                        start=(o == 0), stop=(o == DO - 1),
                                    )
                                for o in range(DO):
                                    nc.tensor.matmul(
                                        ph2[:, :bs],
                                        v1c[:, o, ts(u, P)],
                                        xgT[:, o, ds(b0, bs)],
                                        start=(o == 0), stop=(o == DO - 1),
                                    )
                                g = gpool.tile([P, 512], F32, tag="g", name="g")
                                nc.scalar.activation(g[:, :bs], ph1[:, :bs], AF.Gelu)
                                nc.vector.tensor_tensor(
                                    hts[c][:, u, ds(b0, bs)], g[:, :bs], ph2[:, :bs],
                                    op=OP.mult,
                                )
                    last_pair = cp + CGRP >= NFC
                    for j in range(NJ):
                        for dh in range(2):
                            py = psC.tile([P, 512], F32, tag="y", name="py")
                            for ci, c in enumerate(cs):
                                for u in range(FU):
                                    nc.tensor.matmul(
                                        py[:],
                                        hts[c][:, u, ts(j, P)],
                                        w2s[c][:, u, ts(dh, 512)],
                                        start=(ci == 0 and u == 0),
                                        stop=(ci == len(cs) - 1 and u == FU - 1),
                                    )
                            if cp == 0:
                                nc.vector.tensor_copy(y_sb[:, j, ts(dh, 512)], py[:])
                            else:
                                nc.vector.tensor_tensor(
                                    y_sb[:, j, ts(dh, 512)],
                                    y_sb[:, j, ts(dh, 512)], py[:], op=OP.add,
                                )
                        if last_pair:
                            # finalize + scatter as soon as row j completes
                            nc.vector.scalar_tensor_tensor(
                                y_sb[:, j, :], y_sb[:, j, :], wslot[:, j : j + 1],
                                biasbg[:], op0=OP.mult, op1=OP.add,
                            )
                            nc.gpsimd.indirect_dma_start(
                                out=y_d[:],
                                out_offset=bass.IndirectOffsetOnAxis(
                                    ap=gidx_s[:, j : j + 1], axis=0
                                ),
                                in_=y_sb[:, j, :],
                                in_offset=None,
                            )

            _wcm.__exit__(None, None, None)

    nc.finalize()
    return nc


def make_in_maps(inputs):
    x = np.ascontiguousarray(
        np.asarray(inputs["x"], dtype=np.float32).reshape(T, D)
    )
    rw = np.ascontiguousarray(np.asarray(inputs["router_w"], dtype=np.float32))
    w1 = np.asarray(inputs["w1"], dtype=np.float32)
    v1 = np.asarray(inputs["v1"], dtype=np.float32)
    w2 = np.asarray(inputs["w2"], dtype=np.float32)
    bias = np.asarray(inputs["bias"], dtype=np.float32)

    lstrict = np.triu(np.ones((P, P), dtype=np.float32), 1)
    # token t = f*128 + p lives at mask[p, f]
    tokid = (np.arange(NT)[None, :] * P + np.arange(P)[:, None]).astype(np.float32)
    slotiota = np.tile(np.arange(CPAD, dtype=np.float32)[None, :], (P, 1))

    in_maps = []
    for c in range(E):
        onehot = np.zeros((P, E), dtype=np.float32)
        onehot[:, c] = 1.0
        # runtime zero-inits the output; each token is scattered by exactly
        # TOP_K=2 cores, so each scatter adds bias/2
        biasbg = np.tile(bias[None, :] * 0.5, (P, 1)).astype(np.float32)
        in_maps.append(
            {
                "x": x,
                "rw": rw,
                "w1": np.ascontiguousarray(w1[c]),
                "v1": np.ascontiguousarray(v1[c]),
                "w2": np.ascontiguousarray(w2[c]),
                "onehot": onehot,
                "lstrict": lstrict,
                "tokid": tokid,
                "slotiota": slotiota,
                "biasbg": biasbg,
            }
        )
    return in_maps


_NC_CACHE = {}
last_results = None


def kernel(**inputs) -> np.ndarray:
    global last_results
    from concourse.bass_utils import run_bass_kernel_spmd

    if "nc" not in _NC_CACHE:
        _NC_CACHE["nc"] = build_nc()
    nc = _NC_CACHE["nc"]

    in_maps = make_in_maps(inputs)
    trace = bool(int(os.environ.get("MOE_TRACE", "0")))
    res = run_bass_kernel_spmd(
        nc, in_maps, core_ids=list(range(E)), trace=trace,
        stitch_traces=trace, trace_cores=list(range(E)) if trace else None,
    )
    last_results = res
    out = np.zeros((T, D), dtype=np.float32)
    for r in res.results:
        out += r["y"][:T]
    return out.reshape(2, 2048, D)
